# revision 8
# baseline (speedup 1.0000x reference)
"""2D Haar DWT (analysis) on 8 Trainium2 NeuronCores.

Input  x: (16, 64, 256, 256) f32  -> 1024 independent 256x256 images.
Output: tuple (LL, LH, HL, HH), each (16, 64, 128, 128) f32.

With Haar filters the DWT is a 2x2 butterfly: for each 2x2 block
(a b / c d), with the 0.5 scale folded into a host-side prescale:
    LL = a+b+c+d, LH = a-b+c-d, HL = a+b-c-d, HH = a-b-c+d
i.e. two levels of adds/subs -- no matmul. fp32 matmuls stream at half
rate on the PE and would dominate; plain VectorE adds finish in ~145us
per core, under the ~160us DMA-fabric floor for 67MB of traffic.

Layout (everything unit-stride, partition dim = image index):
  - host prescales x by 0.5 and deinterleaves even/odd columns, then
    flattens each image row-major: xin[img, h*256 + e*128 + w]
  - per core 128 images; the 256 rows are processed in variable-size
    chunks (small chunks at the start/end shrink pipeline fill/drain,
    2MB chunks in the middle keep DMA descriptors big)
  - per chunk: one input DMA, 6 VectorE tensor ops, one output DMA.
"""

import numpy as np

import concourse.bacc as bacc
import concourse.tile as tile
from concourse import mybir
from concourse.bass_utils import run_bass_kernel_spmd

N_CORES = 8
B, C, H, W = 16, 64, 256, 256
N_IMG = B * C                    # 1024
P = N_IMG // N_CORES             # 128 images per core = partition dim
Wh = W // 2                      # 128
# rows per chunk: sum must be H; small head chunks start compute early,
# small tail chunks shrink the drain after the last input lands
CHUNKS = [8, 8] + [16] * 14 + [4] * 4
assert sum(CHUNKS) == H
F32 = mybir.dt.float32

_CACHE = {}


def _build_program():
    nc = bacc.Bacc(
        "TRN2",
        target_bir_lowering=False,
        debug=False,
        enable_asserts=False,
        num_devices=N_CORES,
    )
    xin = nc.dram_tensor("xin", [P, H * W], F32, kind="ExternalInput").ap()
    out = nc.dram_tensor("out", [P, H * W // 4 * 4], F32, kind="ExternalOutput").ap()

    with tile.TileContext(nc) as tc:
        with (
            tc.tile_pool(name="xp", bufs=5) as xp,
            tc.tile_pool(name="mid", bufs=3) as mid,
            tc.tile_pool(name="op", bufs=4) as op,
        ):
            r0 = 0
            o0 = 0
            for hc in CHUNKS:
                isz = hc * W          # input elems per partition
                osz = 4 * (hc // 2) * Wh  # output elems per partition (= isz)
                xt = xp.tile([P, isz], F32, tag="xt")
                nc.sync.dma_start(out=xt, in_=xin[:, r0 * W:(r0 + hc) * W])
                xv = xt.rearrange("p (h e w) -> p h e w", h=hc, e=2, w=Wh)
                xe = xv[:, :, 0, :].rearrange("p (i f) w -> p i f w", f=2)
                xo = xv[:, :, 1, :].rearrange("p (i f) w -> p i f w", f=2)
                # column butterfly: sw/dw[h] = x[h, even] +/- x[h, odd],
                # rows pre-split into (pair, parity) for the row butterfly
                sw = mid.tile([P, hc // 2, 2, Wh], F32, tag="sw")
                dw = mid.tile([P, hc // 2, 2, Wh], F32, tag="dw")
                nc.vector.tensor_add(sw, xe, xo)
                nc.vector.tensor_sub(dw, xe, xo)
                ot = op.tile([P, osz], F32, tag="ot")
                ov = ot.rearrange("p (b i w) -> p b i w", b=4, i=hc // 2, w=Wh)
                nc.vector.tensor_add(ov[:, 0], sw[:, :, 0, :], sw[:, :, 1, :])  # LL
                nc.vector.tensor_add(ov[:, 1], dw[:, :, 0, :], dw[:, :, 1, :])  # LH
                nc.vector.tensor_sub(ov[:, 2], sw[:, :, 0, :], sw[:, :, 1, :])  # HL
                nc.vector.tensor_sub(ov[:, 3], dw[:, :, 0, :], dw[:, :, 1, :])  # HH
                nc.scalar.dma_start(out=out[:, o0:o0 + osz], in_=ot)
                r0 += hc
                o0 += osz
    nc.compile()
    return nc


def kernel(x, m_l0, m_l1, m_h0, m_h1):
    x = np.asarray(x, dtype=np.float32)
    assert x.shape == (B, C, H, W), x.shape

    if "nc" not in _CACHE:
        _CACHE["nc"] = _build_program()
    nc = _CACHE["nc"]

    # prescale by 0.5 (exact) and split even/odd columns: [N, H, 2, W/2]
    xs = (x.reshape(N_IMG, H, W // 2, 2) * np.float32(0.5)).transpose(0, 1, 3, 2)
    in_maps = []
    for s in range(N_CORES):
        shard = xs[s * P:(s + 1) * P]  # [128, 256, 2, 128]
        in_maps.append({"xin": np.ascontiguousarray(shard.reshape(P, H * W))})

    res = run_bass_kernel_spmd(nc, in_maps, core_ids=list(range(N_CORES)))

    parts = []
    for s in range(N_CORES):
        o = res.results[s]["out"]  # [P, sum over chunks of 4*(hc/2)*128]
        img = np.empty((P, 4, H // 2, Wh), np.float32)
        o0 = 0
        r0 = 0
        for hc in CHUNKS:
            osz = 4 * (hc // 2) * Wh
            blk = o[:, o0:o0 + osz].reshape(P, 4, hc // 2, Wh)
            img[:, :, r0 // 2:(r0 + hc) // 2] = blk
            o0 += osz
            r0 += hc
        parts.append(img)
    full = np.concatenate(parts, axis=0).reshape(B, C, 4, H // 2, Wh)
    LL = np.ascontiguousarray(full[:, :, 0])
    LH = np.ascontiguousarray(full[:, :, 1])
    HL = np.ascontiguousarray(full[:, :, 2])
    HH = np.ascontiguousarray(full[:, :, 3])
    return (LL, LH, HL, HH)


# revision 10
# speedup vs baseline: 1.1498x; 1.1498x over previous
"""2D Haar DWT (analysis) on 8 Trainium2 NeuronCores.

Input  x: (16, 64, 256, 256) f32  -> 1024 independent 256x256 images.
Output: tuple (LL, LH, HL, HH), each (16, 64, 128, 128) f32.

With Haar filters the DWT is a 2x2 butterfly: for each 2x2 block
(a b / c d), with the 0.5 scale folded into a host-side prescale:
    LL = a+b+c+d, LH = a-b+c-d, HL = a+b-c-d, HH = a-b-c+d
i.e. two levels of adds/subs -- no matmul. fp32 matmuls stream at half
rate on the PE and would dominate (measured 505us); plain VectorE adds
finish in ~145us per core, under the ~160us DMA-fabric floor for 67MB
of HBM traffic, so the kernel runs at the DMA roofline.

Layout (everything unit-stride, partition dim = image index):
  - host prescales x by 0.5 and deinterleaves even/odd columns
  - per core 128 images; rows processed in chunks; each chunk is one
    fully contiguous DRAM block [img, hc rows] so DMA descriptors are
    maximal (2MB transfers, 16KB/partition runs)
  - small tail chunks shrink the pipeline drain after the last input
  - per chunk: one input DMA, 6 VectorE tensor ops, one output DMA.
"""

import numpy as np

import concourse.bacc as bacc
import concourse.tile as tile
from concourse import mybir
from concourse.bass_utils import run_bass_kernel_spmd

N_CORES = 8
B, C, H, W = 16, 64, 256, 256
N_IMG = B * C                    # 1024
P = N_IMG // N_CORES             # 128 images per core = partition dim
Wh = W // 2                      # 128
HC_BIG, N_BIG = 16, 15           # 15 x 16 rows (2MB chunks)
HC_SM, N_SM = 4, 4               # + 4 x 4 rows (0.5MB tail chunks)
assert HC_BIG * N_BIG + HC_SM * N_SM == H
F32 = mybir.dt.float32

_CACHE = {}


def _butterfly(nc, xt, mid, op, hc):
    """Emit the 6 VectorE ops for one chunk; returns the output tile."""
    xv = xt.rearrange("p (h e w) -> p h e w", h=hc, e=2, w=Wh)
    xe = xv[:, :, 0, :].rearrange("p (i f) w -> p i f w", f=2)
    xo = xv[:, :, 1, :].rearrange("p (i f) w -> p i f w", f=2)
    sw = mid.tile([P, hc // 2, 2, Wh], F32, tag="sw")
    dw = mid.tile([P, hc // 2, 2, Wh], F32, tag="dw")
    nc.vector.tensor_add(sw, xe, xo)
    nc.vector.tensor_sub(dw, xe, xo)
    ot = op.tile([P, 4 * (hc // 2) * Wh], F32, tag="ot")
    ov = ot.rearrange("p (b i w) -> p b i w", b=4, i=hc // 2, w=Wh)
    nc.vector.tensor_add(ov[:, 0], sw[:, :, 0, :], sw[:, :, 1, :])  # LL
    nc.vector.tensor_add(ov[:, 1], dw[:, :, 0, :], dw[:, :, 1, :])  # LH
    nc.vector.tensor_sub(ov[:, 2], sw[:, :, 0, :], sw[:, :, 1, :])  # HL
    nc.vector.tensor_sub(ov[:, 3], dw[:, :, 0, :], dw[:, :, 1, :])  # HH
    return ot


def _build_program():
    nc = bacc.Bacc(
        "TRN2",
        target_bir_lowering=False,
        debug=False,
        enable_asserts=False,
        num_devices=N_CORES,
    )
    xb = nc.dram_tensor("xb", [N_BIG, P, HC_BIG * W], F32, kind="ExternalInput").ap()
    xs = nc.dram_tensor("xs", [N_SM, P, HC_SM * W], F32, kind="ExternalInput").ap()
    ob = nc.dram_tensor("ob", [N_BIG, P, HC_BIG * W], F32, kind="ExternalOutput").ap()
    os_ = nc.dram_tensor("os", [N_SM, P, HC_SM * W], F32, kind="ExternalOutput").ap()

    with tile.TileContext(nc) as tc:
        with (
            tc.tile_pool(name="xp", bufs=5) as xp,
            tc.tile_pool(name="mid", bufs=3) as mid,
            tc.tile_pool(name="op", bufs=4) as op,
        ):
            for k in range(N_BIG):
                xt = xp.tile([P, HC_BIG * W], F32, tag="xt")
                nc.sync.dma_start(out=xt, in_=xb[k])
                ot = _butterfly(nc, xt, mid, op, HC_BIG)
                nc.scalar.dma_start(out=ob[k], in_=ot)
            for k in range(N_SM):
                xt = xp.tile([P, HC_SM * W], F32, tag="xt")
                nc.sync.dma_start(out=xt, in_=xs[k])
                ot = _butterfly(nc, xt, mid, op, HC_SM)
                nc.scalar.dma_start(out=os_[k], in_=ot)
    nc.compile()
    return nc


def kernel(x, m_l0, m_l1, m_h0, m_h1):
    x = np.asarray(x, dtype=np.float32)
    assert x.shape == (B, C, H, W), x.shape

    if "nc" not in _CACHE:
        _CACHE["nc"] = _build_program()
    nc = _CACHE["nc"]

    # prescale by 0.5 (exact) and split even/odd columns: [N, H, 2, W/2]
    xsp = (x.reshape(N_IMG, H, W // 2, 2) * np.float32(0.5)).transpose(0, 1, 3, 2)
    r_split = N_BIG * HC_BIG
    in_maps = []
    for s in range(N_CORES):
        shard = xsp[s * P:(s + 1) * P]  # [128, 256, 2, 128]
        big = shard[:, :r_split].reshape(P, N_BIG, HC_BIG * W).transpose(1, 0, 2)
        sm = shard[:, r_split:].reshape(P, N_SM, HC_SM * W).transpose(1, 0, 2)
        in_maps.append({
            "xb": np.ascontiguousarray(big),
            "xs": np.ascontiguousarray(sm),
        })

    res = run_bass_kernel_spmd(nc, in_maps, core_ids=list(range(N_CORES)))

    parts = []
    for s in range(N_CORES):
        img = np.empty((P, 4, H // 2, Wh), np.float32)
        obig = res.results[s]["ob"].reshape(N_BIG, P, 4, HC_BIG // 2, Wh)
        img[:, :, :r_split // 2] = obig.transpose(1, 2, 0, 3, 4).reshape(
            P, 4, r_split // 2, Wh)
        osm = res.results[s]["os"].reshape(N_SM, P, 4, HC_SM // 2, Wh)
        img[:, :, r_split // 2:] = osm.transpose(1, 2, 0, 3, 4).reshape(
            P, 4, (H - r_split) // 2, Wh)
        parts.append(img)
    full = np.concatenate(parts, axis=0).reshape(B, C, 4, H // 2, Wh)
    LL = np.ascontiguousarray(full[:, :, 0])
    LH = np.ascontiguousarray(full[:, :, 1])
    HL = np.ascontiguousarray(full[:, :, 2])
    HH = np.ascontiguousarray(full[:, :, 3])
    return (LL, LH, HL, HH)
